# revision 7
# baseline (speedup 1.0000x reference)
"""DiffUnpool batched GEMM on 8 Trainium2 NeuronCores.

out[b] = S[b] @ x[b] for b in 0..15 (B=16, M=2048, K=256, N=256); A is
passed through unused and never touches the device.

Sharding: pure data parallel over the batch dim - 2 batches per core, no
communication.  Host side pre-transposes S to S^T[b, p, n] so the
contraction dim p lands on SBUF partitions for both matmul operands (fp32
cannot use the DMA-transpose engine, and on-chip PE transposes would double
TensorE work).

Per-core device kernel (exact fp32, PE-bound at the fp32 4-cycles/row
streaming floor ~27.3us; HW-measured steady-state body ~27.5us):
  - x k-tiles loaded first (small, needed by every matmul),
  - S^T loaded as [128, 512] column chunks (256 KB DMAs measured ~1.5x
    faster end-to-end than monolithic 1 MB loads: earlier PE start, less
    SDMA monopolization), deep prefetch (32 chunk buffers = 2 batches),
  - 16 m-tiles x 2 accumulating matmuls into PSUM [128n, 256c],
  - DVE copy PSUM->SBUF, output stores issued from the ACT queue so the
    SP load queue never head-of-line blocks behind stores waiting on
    copies (HW-measured ~6us/rep win).
"""

import numpy as np

B, N_ORIG, N_POOL, C = 16, 2048, 256, 256
N_CORES = 8
B_PER_CORE = B // N_CORES
WCHUNK = 512

_cache: dict = {}


def _apply_multiwait_split_patch():
    """This walrus build rejects instructions with >1 sync wait (CoreV3
    setupSyncWait: "Too many sync wait commands"), but Tile's add_semaphores
    stage attaches several.  Post-process the serialized BIR: for each
    instruction with N>1 waits insert N-1 single-wait NoOps right before it
    on the same engine - per-engine program order preserves the semantics."""
    import orjson
    import concourse.bass as bass

    if getattr(bass.Bass, "_mwsplit_patched", False):
        return

    counter = [0]

    def split_multiwait(bir: dict) -> dict:
        for fn in bir.get("functions", []):
            for blk in fn.get("blocks", []):
                out = []
                changed = False
                for inst in blk.get("instructions", []):
                    si = inst.get("sync_info") or {}
                    waits = si.get("on_wait") or []
                    if len(waits) > 1:
                        changed = True
                        for w in waits[:-1]:
                            counter[0] += 1
                            out.append(
                                {
                                    "engine": inst["engine"],
                                    "ins": [],
                                    "outs": [],
                                    "name": f"I-mwsplit-{counter[0]}",
                                    "opcode": "NoOp",
                                    "debug": inst.get("debug", 0),
                                    "sync_info": {"on_update": [], "on_wait": [w]},
                                }
                            )
                        si["on_wait"] = [waits[-1]]
                    out.append(inst)
                if changed:
                    blk["instructions"] = out
        return bir

    orig_bytes = bass.Bass.to_json_bytes

    def to_json_bytes(self) -> bytes:
        return orjson.dumps(split_multiwait(orjson.loads(orig_bytes(self))))

    def to_json_str(self) -> str:
        return to_json_bytes(self).decode()

    def to_json(self) -> dict:
        return orjson.loads(to_json_bytes(self))

    bass.Bass.to_json_bytes = to_json_bytes
    bass.Bass.to_json_str = to_json_str
    bass.Bass.to_json = to_json
    bass.Bass._mwsplit_patched = True


def _build_nc(reps: int = 1):
    import concourse.bass as bass
    import concourse.mybir as mybir
    import concourse.tile as tile

    _apply_multiwait_split_patch()

    f32 = mybir.dt.float32
    nc = bass.Bass()
    # Per-core: st = S^T slices [b, p, n], xs = x slices, out = S @ x.
    st = nc.declare_dram_parameter(
        "st", [B_PER_CORE, N_POOL, N_ORIG], f32, isOutput=False
    )
    xs = nc.declare_dram_parameter("xs", [B_PER_CORE, N_POOL, C], f32, isOutput=False)
    out = nc.declare_dram_parameter(
        "out", [B_PER_CORE, N_ORIG, C], f32, isOutput=True
    )

    KT = N_POOL // 128      # k-tiles per batch (2)
    MT = N_ORIG // 128      # m-tiles per batch (16)
    NCH = N_ORIG // WCHUNK  # weight column chunks per k-tile (4)
    MPC = WCHUNK // 128     # m-tiles covered per chunk (4)

    with tile.TileContext(nc) as tc:
        with (
            tc.tile_pool(name="w", bufs=2 * KT * NCH) as wpool,
            tc.tile_pool(name="xp", bufs=2 * KT) as xpool,
            tc.tile_pool(name="ps", bufs=7, space="PSUM") as pspool,
            tc.tile_pool(name="wps", bufs=1, space="PSUM") as wpspool,
            tc.tile_pool(name="ob", bufs=6) as opool,
            tc.tile_pool(name="wu", bufs=1) as wupool,
        ):
            # PE warmup: dummy matmuls into a scratch PSUM bank while the
            # first input DMAs are in flight, so the HAM clock-gate ramp
            # (cold 1.2 GHz -> warm 2.4 GHz) burns off before real matmuls.
            dummy_w = wupool.tile([128, 128], f32, tag="wu_w")
            dummy_x = wupool.tile([128, 64], f32, tag="wu_x")
            nc.gpsimd.memset(dummy_w[:], 1.0)
            nc.gpsimd.memset(dummy_x[:], 1.0)
            wps = wpspool.tile([128, 64], f32)
            NWU = 16
            for i in range(NWU):
                nc.tensor.matmul(
                    wps[:], dummy_w[:], dummy_x[:], start=(i == 0), stop=(i == NWU - 1)
                )
            for _ in range(reps):
                for b in range(B_PER_CORE):
                    wc = {}
                    xt = []
                    for k in range(KT):
                        xk = xpool.tile([128, C], f32, tag="x")
                        nc.sync.dma_start(
                            out=xk[:], in_=xs[b, k * 128 : (k + 1) * 128, :]
                        )
                        xt.append(xk)
                    for k in range(KT):
                        for ch in range(NCH):
                            w = wpool.tile([128, WCHUNK], f32, tag="w")
                            nc.sync.dma_start(
                                out=w[:],
                                in_=st[
                                    b,
                                    k * 128 : (k + 1) * 128,
                                    ch * WCHUNK : (ch + 1) * WCHUNK,
                                ],
                            )
                            wc[(k, ch)] = w
                    for m in range(MT):
                        ch, off = divmod(m, MPC)
                        ps = pspool.tile([128, C], f32, tag="ps")
                        for k in range(KT):
                            nc.tensor.matmul(
                                ps[:],
                                wc[(k, ch)][:, off * 128 : (off + 1) * 128],
                                xt[k][:],
                                start=(k == 0),
                                stop=(k == KT - 1),
                            )
                        ob = opool.tile([128, C], f32, tag="ob")
                        nc.vector.tensor_copy(ob[:], ps[:])
                        # stores on the ACT HWDGE queue: keeps the SP queue
                        # free for loads (in-order issue would head-of-line
                        # block the next batch's loads behind stores).
                        nc.scalar.dma_start(
                            out=out[b, m * 128 : (m + 1) * 128, :], in_=ob[:]
                        )
    return nc


def _get_nc():
    if "nc" not in _cache:
        _cache["nc"] = _build_nc()
    return _cache["nc"]


def _run(x: np.ndarray, S: np.ndarray, trace: bool = False):
    from concourse.bass_utils import run_bass_kernel_spmd

    nc = _get_nc()
    st_full = np.ascontiguousarray(S.transpose(0, 2, 1))
    x_full = np.ascontiguousarray(x)
    core_ids = list(range(N_CORES))
    in_maps = [
        {
            "st": st_full[i * B_PER_CORE : (i + 1) * B_PER_CORE],
            "xs": x_full[i * B_PER_CORE : (i + 1) * B_PER_CORE],
        }
        for i in core_ids
    ]
    res = run_bass_kernel_spmd(nc, in_maps, core_ids, trace=trace)
    out = np.concatenate([res.results[i]["out"] for i in core_ids], axis=0)
    return out, res


def kernel(x: np.ndarray, S: np.ndarray, A: np.ndarray = None, **_: dict) -> np.ndarray:
    x = np.asarray(x, dtype=np.float32)
    S = np.asarray(S, dtype=np.float32)
    out, _res = _run(x, S, trace=False)
    return out
